# revision 14
# baseline (speedup 1.0000x reference)
"""Causal self-attention (B=2, T=2048, C=2048, H=16, D=128) on 8 trn2 NeuronCores.

Sharding: tensor-parallel over heads — 2 heads per core. Each core computes
q/k/v projections for its heads, RoPE, causal softmax attention, and a
partial output projection (rows of w_out for its heads). Host sums the 8
partial outputs.

Device layout choices (see comments inline):
  * x is passed transposed (xT [C, B*T]) so the contraction dim C lands on
    SBUF partitions naturally.
  * q/k are kept transposed per head: [D=128 partitions, B*T free].
  * scores are computed transposed (S^T = k-block^T . q) so the second
    matmul (v^T @ P^T) needs no transpose; softmax sums over the partition
    axis are taken with an all-ones stationary matmul accumulated in PSUM.
  * no max-subtraction in softmax (scores are O(1), exp is safe in fp32).
  * all matmuls run as float32r (full-speed fp32 path on the PE).
"""

import math
import sys

import numpy as np

try:
    import concourse.bass as bass  # noqa: F401
except ImportError:
    sys.path.insert(0, "/opt/trn_rl_repo")

import concourse.bass as bass
import concourse.mybir as mybir
from concourse import bacc
import concourse.tile as tile
from concourse.bass_utils import run_bass_kernel_spmd

# Problem dims (hardcoded per harness contract)
B, T, C = 2, 2048, 2048
H = 16
D = 128
N_CORES = 8
HEADS_PER_CORE = H // N_CORES  # 2
ROPE_BASE = 10000.0

BT = B * T  # 4096
TQ = 512  # query tile (matmul moving free dim)
TK = 128  # key block (stationary operand columns)
KC = C // 128  # 16 contraction subtiles for the qkv projection

F32 = mybir.dt.float32
F32R = mybir.dt.float32r

# set by test harness to collect a profile
TRACE = False
LAST_RESULTS = None


def r(ap):
    """View an AP as float32r for full-rate fp32 matmul."""
    return ap.bitcast(F32R)


def build_nc():
    nc = bacc.Bacc()

    xT = nc.dram_tensor("xT", [C, BT], F32, kind="ExternalInput").ap()
    wqkv = nc.dram_tensor("wqkv", [C, 6 * D], F32, kind="ExternalInput").ap()
    wout = nc.dram_tensor("wout", [2 * D, C], F32, kind="ExternalInput").ap()
    cosT = nc.dram_tensor("cosT", [D, T], F32, kind="ExternalInput").ap()
    sinT = nc.dram_tensor("sinT", [D, T], F32, kind="ExternalInput").ap()
    rotT = nc.dram_tensor("rotT", [D, D], F32, kind="ExternalInput").ap()
    tri = nc.dram_tensor("tri", [128, 128], F32, kind="ExternalInput").ap()
    onesm = nc.dram_tensor("onesm", [128, 128], F32, kind="ExternalInput").ap()
    ident = nc.dram_tensor("ident", [128, 128], F32, kind="ExternalInput").ap()
    out = nc.dram_tensor("out", [BT, C], F32, kind="ExternalOutput").ap()

    scale = 1.0 / math.sqrt(D)
    NH = HEADS_PER_CORE
    NQT = T // TQ  # 4 query tiles per (b, h)
    NBT = BT // 128  # 32 row-blocks of BT

    with tile.TileContext(nc) as tc:
        with (
            tc.tile_pool(name="consts", bufs=1) as consts,
            tc.tile_pool(name="persist", bufs=1) as persist,
        ):
            # ---- constants resident for the whole kernel ----
            cos_sb = consts.tile([D, T], F32)
            nc.sync.dma_start(out=cos_sb, in_=cosT)
            sin_sb = consts.tile([D, T], F32)
            nc.sync.dma_start(out=sin_sb, in_=sinT)
            rot_sb = consts.tile([D, D], F32R)
            nc.sync.dma_start(out=rot_sb, in_=r(rotT))
            tri_sb = consts.tile([128, 128], F32)
            nc.sync.dma_start(out=tri_sb, in_=tri)
            ones_sb = consts.tile([128, 128], F32R)
            nc.sync.dma_start(out=ones_sb, in_=r(onesm))
            id_sb = consts.tile([128, 128], F32R)
            nc.sync.dma_start(out=id_sb, in_=r(ident))

            # ---- persistent activations ----
            qT_sb = persist.tile([D, NH, BT], F32R)  # q^T per head (RoPEd)
            kT_sb = persist.tile([D, NH, BT], F32R)  # k^T per head (RoPEd)
            # v in natural orientation: [128 (bt within block), head, block, d]
            vN_sb = persist.tile([128, NH, NBT, D], F32R)

            # =============================================================
            # Phase 1: qkv projection + RoPE + v transpose
            # =============================================================
            with (
                tc.tile_pool(name="p1_w", bufs=1) as p1_w,
                tc.tile_pool(name="p1_x", bufs=3) as p1_x,
                tc.tile_pool(name="p1_t", bufs=2) as p1_t,
                tc.tile_pool(name="p1_ps", bufs=1, space="PSUM") as p1_ps,
                tc.tile_pool(name="p1_ps2", bufs=2, space="PSUM") as p1_ps2,
            ):
                w_sb = p1_w.tile([128, KC, 6 * D], F32R)
                nc.sync.dma_start(
                    out=w_sb, in_=r(wqkv.rearrange("(ks p) c -> p ks c", p=128))
                )

                for n in range(BT // TQ):  # 8 column tiles of B*T
                    t0 = (n * TQ) % T  # position within the batch row
                    # one PSUM bank per output: q0,q1,k0,k1,v0,v1
                    ps = [
                        p1_ps.tile([128, TQ], F32, tag=f"qkvps{j}", name=f"qkvps{j}")
                        for j in range(6)
                    ]
                    for k in range(KC):
                        xn = p1_x.tile([128, TQ], F32R)
                        nc.sync.dma_start(
                            out=xn,
                            in_=r(xT[k * 128 : (k + 1) * 128, n * TQ : (n + 1) * TQ]),
                        )
                        for j in range(6):
                            nc.tensor.matmul(
                                ps[j],
                                w_sb[:, k, j * D : (j + 1) * D],
                                xn,
                                start=(k == 0),
                                stop=(k == KC - 1),
                            )
                    # q and k: RoPE, into qT_sb/kT_sb
                    for h in range(NH):
                        for which, dest in ((0, qT_sb), (1, kT_sb)):
                            src_ps = ps[which * NH + h]
                            raw = p1_t.tile([128, TQ], F32R, tag="raw")
                            nc.scalar.copy(out=raw, in_=src_ps)
                            rot_ps = p1_ps2.tile([128, TQ], F32, tag="scratch_ps")
                            nc.tensor.matmul(
                                rot_ps, rot_sb, raw, start=True, stop=True
                            )
                            qcos = p1_t.tile([128, TQ], F32, tag="qcos")
                            nc.vector.tensor_mul(
                                out=qcos, in0=raw, in1=cos_sb[:, t0 : t0 + TQ]
                            )
                            sinrot = p1_t.tile([128, TQ], F32, tag="sinrot")
                            nc.vector.tensor_mul(
                                out=sinrot, in0=rot_ps, in1=sin_sb[:, t0 : t0 + TQ]
                            )
                            nc.vector.tensor_add(
                                out=dest[:, h, n * TQ : (n + 1) * TQ],
                                in0=qcos,
                                in1=sinrot,
                            )
                    # v: copy to sbuf, then transpose 128x128 blocks to natural layout
                    for h in range(NH):
                        vtile = p1_t.tile([128, TQ], F32R, tag="vtile")
                        nc.scalar.copy(out=vtile, in_=ps[4 + h])
                        for s4 in range(TQ // 128):
                            blk = n * (TQ // 128) + s4
                            tp = p1_ps2.tile([128, TQ], F32, tag="scratch_ps")
                            nc.tensor.transpose(
                                r(tp[:, :128]),
                                vtile[:, s4 * 128 : (s4 + 1) * 128],
                                id_sb,
                            )
                            nc.vector.tensor_copy(
                                out=vN_sb[:, h, blk, :], in_=tp[:, :128]
                            )

            # =============================================================
            # Phase 2: causal attention per (b, h)
            # =============================================================
            # yT_sb lives from phase 2 through phase 3 (pool closed at the end)
            persist2_cm = tc.tile_pool(name="persist2", bufs=1)
            persist2 = persist2_cm.__enter__()
            yT_sb = persist2.tile([D, NH, BT], F32R)  # attention out, transposed
            with (
                tc.tile_pool(name="p2_pt", bufs=4) as p2_pt,
                tc.tile_pool(name="p2_t", bufs=2) as p2_t,
                tc.tile_pool(name="p2_sps", bufs=3, space="PSUM") as p2_sps,
                tc.tile_pool(name="p2_yac", bufs=2, space="PSUM") as p2_yac,
                tc.tile_pool(name="p2_srp", bufs=2, space="PSUM") as p2_srp,
            ):
                for b in range(B):
                    for h in range(NH):
                        for qi in range(NQT):
                            i0 = qi * TQ
                            bt_i = b * T + i0
                            yps = p2_yac.tile([128, TQ], F32, tag="yacc")
                            sps = p2_srp.tile([128, TQ], F32, tag="srep")
                            nblocks = qi * (TQ // 128) + (TQ // 128)
                            for jb in range(nblocks):
                                j0 = jb * 128
                                bt_j = b * T + j0
                                mdiag = jb - qi * (TQ // 128)  # >=0 on diag band
                                c0 = 128 * mdiag if mdiag > 0 else 0
                                first = jb == 0
                                last = jb == nblocks - 1
                                sp = p2_sps.tile([128, TQ], F32, tag="sps")
                                nc.tensor.matmul(
                                    sp[:, c0:],
                                    kT_sb[:, h, bt_j : bt_j + 128],
                                    qT_sb[:, h, bt_i + c0 : bt_i + TQ],
                                    start=True,
                                    stop=True,
                                )
                                pt = p2_pt.tile([128, TQ], F32R, tag="pt")
                                nc.scalar.activation(
                                    out=pt[:, c0:],
                                    in_=sp[:, c0:],
                                    func=mybir.ActivationFunctionType.Exp,
                                    scale=scale,
                                )
                                if mdiag >= 0:
                                    nc.vector.tensor_mul(
                                        out=pt[:, c0 : c0 + 128],
                                        in0=pt[:, c0 : c0 + 128],
                                        in1=tri_sb,
                                    )
                                nc.tensor.matmul(
                                    yps[:, c0:],
                                    vN_sb[:, h, bt_j // 128, :],
                                    pt[:, c0:],
                                    start=first,
                                    stop=last,
                                )
                                nc.tensor.matmul(
                                    sps[:, c0:],
                                    ones_sb,
                                    pt[:, c0:],
                                    start=first,
                                    stop=last,
                                )
                            # normalize: y = y~ * exp(-log(sum))
                            lg = p2_t.tile([128, TQ], F32, tag="lg")
                            nc.scalar.activation(
                                out=lg,
                                in_=sps,
                                func=mybir.ActivationFunctionType.Ln,
                            )
                            rr = p2_t.tile([128, TQ], F32, tag="rr")
                            nc.scalar.activation(
                                out=rr,
                                in_=lg,
                                func=mybir.ActivationFunctionType.Exp,
                                scale=-1.0,
                            )
                            nc.vector.tensor_mul(
                                out=yT_sb[:, h, bt_i : bt_i + TQ],
                                in0=yps,
                                in1=rr,
                            )

            # =============================================================
            # Phase 3: partial output projection: out = y @ w_out_shard
            # =============================================================
            with (
                tc.tile_pool(name="p3_w", bufs=1) as p3_w,
                tc.tile_pool(name="p3_o", bufs=3) as p3_o,
                tc.tile_pool(name="p3_ps", bufs=4, space="PSUM") as p3_ps,
            ):
                wout_sb = p3_w.tile([128, 2, C], F32R)
                nc.sync.dma_start(
                    out=wout_sb, in_=r(wout.rearrange("(s p) c -> p s c", p=128))
                )
                CT = min(TQ, C)  # output-column tile
                for blk in range(NBT):
                    bt0 = blk * 128
                    otile = p3_o.tile([128, C], F32, tag="otile")
                    for ntc in range(C // CT):
                        ops = p3_ps.tile([128, CT], F32, tag="ops")
                        for h in range(NH):
                            nc.tensor.matmul(
                                ops,
                                yT_sb[:, h, bt0 : bt0 + 128],
                                wout_sb[:, h, ntc * CT : (ntc + 1) * CT],
                                start=(h == 0),
                                stop=(h == NH - 1),
                            )
                        if ntc % 2 == 0:
                            nc.vector.tensor_copy(
                                out=otile[:, ntc * CT : (ntc + 1) * CT], in_=ops
                            )
                        else:
                            nc.scalar.copy(
                                out=otile[:, ntc * CT : (ntc + 1) * CT], in_=ops
                            )
                    nc.sync.dma_start(out=out[bt0 : bt0 + 128, :], in_=otile)

            persist2_cm.__exit__(None, None, None)

    nc.finalize()
    return nc


def _host_inputs(x, w_qkv, w_out):
    """Host-side prep: transpose x, slice weights per core, RoPE tables."""
    x = np.asarray(x, dtype=np.float32)
    w_qkv = np.asarray(w_qkv, dtype=np.float32)
    w_out = np.asarray(w_out, dtype=np.float32)

    xT = np.ascontiguousarray(x.reshape(BT, C).T)  # [C, BT]

    # RoPE tables, transposed to [D, T]
    inv_freq = 1.0 / (
        ROPE_BASE ** (np.arange(0, D, 2, dtype=np.float32) / D)
    )  # [64]
    t = np.arange(T, dtype=np.float32)
    freqs = np.outer(t, inv_freq)  # [T, 64]
    emb = np.concatenate([freqs, freqs], axis=-1)  # [T, D]
    cosT = np.ascontiguousarray(np.cos(emb).T.astype(np.float32))  # [D, T]
    sinT = np.ascontiguousarray(np.sin(emb).T.astype(np.float32))

    # rotate_half as a matmul: rot = Rot @ q, lhsT = Rot^T
    rot = np.zeros((D, D), dtype=np.float32)
    half = D // 2
    for dd in range(half):
        rot[dd, dd + half] = -1.0
        rot[dd + half, dd] = 1.0
    rotT = np.ascontiguousarray(rot.T)

    jj, kk = np.meshgrid(np.arange(128), np.arange(128), indexing="ij")
    tri = (jj <= kk).astype(np.float32)  # keep key_local <= query_local
    onesm = np.ones((128, 128), dtype=np.float32)
    ident = np.eye(128, dtype=np.float32)

    in_maps = []
    for c in range(N_CORES):
        h0 = c * HEADS_PER_CORE
        cols = slice(h0 * D, (h0 + HEADS_PER_CORE) * D)
        wq = w_qkv[:, 0 * C :][:, cols]
        wk = w_qkv[:, 1 * C :][:, cols]
        wv = w_qkv[:, 2 * C :][:, cols]
        wqkv_c = np.ascontiguousarray(np.concatenate([wq, wk, wv], axis=1))
        wout_c = np.ascontiguousarray(w_out[cols, :])
        in_maps.append(
            {
                "xT": xT,
                "wqkv": wqkv_c,
                "wout": wout_c,
                "cosT": cosT,
                "sinT": sinT,
                "rotT": rotT,
                "tri": tri,
                "onesm": onesm,
                "ident": ident,
            }
        )
    return in_maps


def kernel(x, w_qkv, w_out):
    global LAST_RESULTS
    in_maps = _host_inputs(x, w_qkv, w_out)
    nc = build_nc()
    res = run_bass_kernel_spmd(
        nc, in_maps, core_ids=list(range(N_CORES)), trace=TRACE
    )
    LAST_RESULTS = res
    acc = np.zeros((BT, C), dtype=np.float64)
    for cr in res.results:
        acc += cr["out"].astype(np.float64)
    return acc.astype(np.float32).reshape(B, T, C)


# revision 22
# speedup vs baseline: 1.1757x; 1.1757x over previous
"""Causal self-attention (B=2, T=2048, C=2048, H=16, D=128) on 8 trn2 NeuronCores.

Sharding: tensor-parallel over heads — 2 heads per core. Each core computes
q/k/v projections for its heads, RoPE, causal softmax attention, and a
partial output projection (rows of w_out for its heads). Host sums the 8
partial outputs.

Device layout choices (see comments inline):
  * x is passed transposed (xT [C, B*T]) so the contraction dim C lands on
    SBUF partitions naturally.
  * q/k are kept transposed per head: [D=128 partitions, B*T free].
  * scores are computed transposed (S^T = k-block^T . q) so the second
    matmul (v^T @ P^T) needs no transpose; softmax sums over the partition
    axis are taken with an all-ones stationary matmul accumulated in PSUM.
  * no max-subtraction in softmax (scores are O(1), exp is safe in fp32).
  * all matmuls run as float32r (full-speed fp32 path on the PE).
"""

import math
import sys

import numpy as np

try:
    import concourse.bass as bass  # noqa: F401
except ImportError:
    sys.path.insert(0, "/opt/trn_rl_repo")

import concourse.bass as bass
import concourse.mybir as mybir
from concourse import bacc
import concourse.tile as tile
from concourse.bass_utils import run_bass_kernel_spmd

# Problem dims (hardcoded per harness contract)
B, T, C = 2, 2048, 2048
H = 16
D = 128
N_CORES = 8
HEADS_PER_CORE = H // N_CORES  # 2
ROPE_BASE = 10000.0

BT = B * T  # 4096
TQ = 512  # query tile (matmul moving free dim)
TK = 128  # key block (stationary operand columns)
KC = C // 128  # 16 contraction subtiles for the qkv projection

F32 = mybir.dt.float32
F32R = mybir.dt.float32r

# set by test harness to collect a profile
TRACE = False
LAST_RESULTS = None


def r(ap):
    """View an AP as float32r for full-rate fp32 matmul."""
    return ap.bitcast(F32R)


def _patch_act_tables():
    """Steer Bacc's ACT-table-set choice to the set holding BOTH Exp and Ln.

    By default Exp resolves to 'exp_and_others' and Ln to 'natural_log', so a
    kernel alternating them reloads tables (~1.3us) every switch. Hiding
    Exp/Ln from every other set forces 'natural_log_exp_and_others' — one
    load for the whole kernel. Table indices are untouched, so the emitted
    act_func_set_id stays consistent with act_info.json.
    """
    import concourse.bacc as bacc_mod

    if getattr(bacc_mod, "_act_tables_patched", False):
        return
    orig = bacc_mod.get_activation_tables

    def patched(arch):
        tables = orig(arch)
        if "natural_log_exp_and_others" in tables:
            exp = mybir.ActivationFunctionType.Exp
            ln = mybir.ActivationFunctionType.Ln
            for name, funcs in tables.items():
                if name != "natural_log_exp_and_others":
                    funcs.discard(exp)
                    funcs.discard(ln)
        return tables

    bacc_mod.get_activation_tables = patched
    bacc_mod._act_tables_patched = True


def build_nc():
    _patch_act_tables()
    nc = bacc.Bacc()

    xT = nc.dram_tensor("xT", [C, BT], F32, kind="ExternalInput").ap()
    wqkv = nc.dram_tensor("wqkv", [C, 6 * D], F32, kind="ExternalInput").ap()
    wout = nc.dram_tensor("wout", [2 * D, C], F32, kind="ExternalInput").ap()
    cosT = nc.dram_tensor("cosT", [D, T], F32, kind="ExternalInput").ap()
    sinT = nc.dram_tensor("sinT", [D, T], F32, kind="ExternalInput").ap()
    tri = nc.dram_tensor("tri", [128, 128], F32, kind="ExternalInput").ap()
    onesm = nc.dram_tensor("onesm", [128, 128], F32, kind="ExternalInput").ap()
    ident = nc.dram_tensor("ident", [128, 128], F32, kind="ExternalInput").ap()
    out = nc.dram_tensor("out", [BT, C], F32, kind="ExternalOutput").ap()

    scale = 1.0 / math.sqrt(D)
    NH = HEADS_PER_CORE
    NQT = T // TQ  # 4 query tiles per (b, h)
    NBT = BT // 128  # 32 row-blocks of BT

    with tile.TileContext(nc) as tc:
        with (
            tc.tile_pool(name="consts", bufs=1) as consts,
            tc.tile_pool(name="persist", bufs=1) as persist,
        ):
            # ---- constants resident for the whole kernel ----
            cos_sb = consts.tile([D, T], F32)
            nc.sync.dma_start(out=cos_sb, in_=cosT)
            sin_sb = consts.tile([D, T], F32)
            nc.sync.dma_start(out=sin_sb, in_=sinT)
            tri_sb = consts.tile([128, 128], F32)
            nc.sync.dma_start(out=tri_sb, in_=tri)
            ones_sb = consts.tile([128, 128], F32R)
            nc.sync.dma_start(out=ones_sb, in_=r(onesm))
            id_sb = consts.tile([128, 128], F32R)
            nc.sync.dma_start(out=id_sb, in_=r(ident))

            # ---- persistent activations ----
            qT_sb = persist.tile([D, NH, BT], F32R)  # q^T per head (RoPEd)
            kT_sb = persist.tile([D, NH, BT], F32R)  # k^T per head (RoPEd)
            # v in natural orientation: [128 (bt within block), head, block, d]
            vN_sb = persist.tile([128, NH, NBT, D], F32R)

            # =============================================================
            # Phase 1: qkv projection + RoPE + v transpose
            # =============================================================
            with (
                tc.tile_pool(name="p1_w", bufs=1) as p1_w,
                tc.tile_pool(name="p1_x", bufs=3) as p1_x,
                tc.tile_pool(name="p1_t", bufs=2) as p1_t,
                tc.tile_pool(name="p1_ps", bufs=1, space="PSUM") as p1_ps,
                tc.tile_pool(name="p1_ps2", bufs=2, space="PSUM") as p1_ps2,
            ):
                w_sb = p1_w.tile([128, KC, 6 * D], F32R)
                nc.sync.dma_start(
                    out=w_sb, in_=r(wqkv.rearrange("(ks p) c -> p ks c", p=128))
                )

                for n in range(BT // TQ):  # 8 column tiles of B*T
                    t0 = (n * TQ) % T  # position within the batch row
                    # one PSUM bank per output: q0,q1,k0,k1,v0,v1
                    ps = [
                        p1_ps.tile([128, TQ], F32, tag=f"qkvps{j}", name=f"qkvps{j}")
                        for j in range(6)
                    ]
                    for k in range(KC):
                        xn = p1_x.tile([128, TQ], F32R)
                        nc.sync.dma_start(
                            out=xn,
                            in_=r(xT[k * 128 : (k + 1) * 128, n * TQ : (n + 1) * TQ]),
                        )
                        for j in range(6):
                            nc.tensor.matmul(
                                ps[j],
                                w_sb[:, k, j * D : (j + 1) * D],
                                xn,
                                start=(k == 0),
                                stop=(k == KC - 1),
                            )
                    # q and k: RoPE, into qT_sb/kT_sb. rotate_half is a
                    # cross-partition move -> two SBUF->SBUF DMAs; sin_sb is
                    # sign-folded on the host so no negation is needed.
                    for h in range(NH):
                        for which, dest in ((0, qT_sb), (1, kT_sb)):
                            src_ps = ps[which * NH + h]
                            raw = p1_t.tile([128, TQ], F32, tag="raw", name="raw")
                            # alternate engines so psum eviction isn't serial
                            if (h + which) % 2 == 0:
                                nc.scalar.copy(out=raw, in_=src_ps)
                            else:
                                nc.vector.tensor_copy(out=raw, in_=src_ps)
                            rot = p1_t.tile([128, TQ], F32, tag="rot", name="rot")
                            nc.sync.dma_start(out=rot[0:64, :], in_=raw[64:128, :])
                            nc.sync.dma_start(out=rot[64:128, :], in_=raw[0:64, :])
                            qcos = p1_t.tile([128, TQ], F32, tag="qcos")
                            nc.vector.tensor_mul(
                                out=qcos, in0=raw, in1=cos_sb[:, t0 : t0 + TQ]
                            )
                            nc.vector.tensor_mul(
                                out=rot, in0=rot, in1=sin_sb[:, t0 : t0 + TQ]
                            )
                            nc.vector.tensor_add(
                                out=dest[:, h, n * TQ : (n + 1) * TQ],
                                in0=qcos,
                                in1=rot,
                            )
                    # v: copy to sbuf, then transpose 128x128 blocks to natural layout
                    for h in range(NH):
                        vtile = p1_t.tile([128, TQ], F32R, tag="vtile")
                        nc.scalar.copy(out=vtile, in_=ps[4 + h])
                        for s4 in range(TQ // 128):
                            blk = n * (TQ // 128) + s4
                            tp = p1_ps2.tile([128, TQ], F32, tag="scratch_ps")
                            nc.tensor.transpose(
                                r(tp[:, :128]),
                                vtile[:, s4 * 128 : (s4 + 1) * 128],
                                id_sb,
                            )
                            nc.vector.tensor_copy(
                                out=vN_sb[:, h, blk, :], in_=tp[:, :128]
                            )

            # =============================================================
            # Phase 2: causal attention per (b, h)
            # =============================================================
            # yT_sb lives from phase 2 through phase 3 (pool closed at the end)
            persist2_cm = tc.tile_pool(name="persist2", bufs=1)
            persist2 = persist2_cm.__enter__()
            yT_sb = persist2.tile([D, NH, BT], F32R)  # attention out, transposed
            # prefetch w_out during phase 2 so phase 3 starts without a stall
            wout_sb = persist2.tile([128, 2, C], F32R)
            nc.sync.dma_start(
                out=wout_sb, in_=r(wout.rearrange("(s p) c -> p s c", p=128))
            )
            with (
                tc.tile_pool(name="p2_pt", bufs=6) as p2_pt,
                tc.tile_pool(name="p2_t", bufs=2) as p2_t,
                tc.tile_pool(name="p2_sps", bufs=4, space="PSUM") as p2_sps,
                tc.tile_pool(name="p2_yac", bufs=2, space="PSUM") as p2_yac,
                tc.tile_pool(name="p2_srp", bufs=2, space="PSUM") as p2_srp,
            ):
                for b in range(B):
                    for h in range(NH):
                        for qi in range(NQT):
                            i0 = qi * TQ
                            bt_i = b * T + i0
                            yps = p2_yac.tile([128, TQ], F32, tag="yacc")
                            sps = p2_srp.tile([128, TQ], F32, tag="srep")
                            nblocks = qi * (TQ // 128) + (TQ // 128)
                            for jb in range(nblocks):
                                j0 = jb * 128
                                bt_j = b * T + j0
                                mdiag = jb - qi * (TQ // 128)  # >=0 on diag band
                                c0 = 128 * mdiag if mdiag > 0 else 0
                                first = jb == 0
                                last = jb == nblocks - 1
                                sp = p2_sps.tile([128, TQ], F32, tag="sps")
                                nc.tensor.matmul(
                                    sp[:, c0:],
                                    kT_sb[:, h, bt_j : bt_j + 128],
                                    qT_sb[:, h, bt_i + c0 : bt_i + TQ],
                                    start=True,
                                    stop=True,
                                )
                                pt = p2_pt.tile([128, TQ], F32R, tag="pt")
                                nc.scalar.activation(
                                    out=pt[:, c0:],
                                    in_=sp[:, c0:],
                                    func=mybir.ActivationFunctionType.Exp,
                                    scale=scale,
                                )
                                if mdiag >= 0:
                                    nc.vector.tensor_mul(
                                        out=pt[:, c0 : c0 + 128],
                                        in0=pt[:, c0 : c0 + 128],
                                        in1=tri_sb,
                                    )
                                nc.tensor.matmul(
                                    yps[:, c0:],
                                    vN_sb[:, h, bt_j // 128, :],
                                    pt[:, c0:],
                                    start=first,
                                    stop=last,
                                )
                                nc.tensor.matmul(
                                    sps[:, c0:],
                                    ones_sb,
                                    pt[:, c0:],
                                    start=first,
                                    stop=last,
                                )
                            # normalize: y = y~ * exp(-log(sum))
                            lg = p2_t.tile([128, TQ], F32, tag="lg")
                            nc.scalar.activation(
                                out=lg,
                                in_=sps,
                                func=mybir.ActivationFunctionType.Ln,
                            )
                            rr = p2_t.tile([128, TQ], F32, tag="rr")
                            nc.scalar.activation(
                                out=rr,
                                in_=lg,
                                func=mybir.ActivationFunctionType.Exp,
                                scale=-1.0,
                            )
                            nc.vector.tensor_mul(
                                out=yT_sb[:, h, bt_i : bt_i + TQ],
                                in0=yps,
                                in1=rr,
                            )

            # =============================================================
            # Phase 3: partial output projection: out = y @ w_out_shard
            # =============================================================
            with (
                tc.tile_pool(name="p3_o", bufs=3) as p3_o,
                tc.tile_pool(name="p3_ps", bufs=4, space="PSUM") as p3_ps,
            ):
                CT = min(TQ, C)  # output-column tile
                for blk in range(NBT):
                    bt0 = blk * 128
                    otile = p3_o.tile([128, C], F32, tag="otile")
                    for ntc in range(C // CT):
                        ops = p3_ps.tile([128, CT], F32, tag="ops")
                        for h in range(NH):
                            nc.tensor.matmul(
                                ops,
                                yT_sb[:, h, bt0 : bt0 + 128],
                                wout_sb[:, h, ntc * CT : (ntc + 1) * CT],
                                start=(h == 0),
                                stop=(h == NH - 1),
                            )
                        if ntc % 2 == 0:
                            nc.vector.tensor_copy(
                                out=otile[:, ntc * CT : (ntc + 1) * CT], in_=ops
                            )
                        else:
                            nc.scalar.copy(
                                out=otile[:, ntc * CT : (ntc + 1) * CT], in_=ops
                            )
                    nc.sync.dma_start(out=out[bt0 : bt0 + 128, :], in_=otile)

            persist2_cm.__exit__(None, None, None)

    nc.finalize()
    return nc


def _host_inputs(x, w_qkv, w_out):
    """Host-side prep: transpose x, slice weights per core, RoPE tables."""
    x = np.asarray(x, dtype=np.float32)
    w_qkv = np.asarray(w_qkv, dtype=np.float32)
    w_out = np.asarray(w_out, dtype=np.float32)

    xT = np.ascontiguousarray(x.reshape(BT, C).T)  # [C, BT]

    # RoPE tables, transposed to [D, T]
    inv_freq = 1.0 / (
        ROPE_BASE ** (np.arange(0, D, 2, dtype=np.float32) / D)
    )  # [64]
    t = np.arange(T, dtype=np.float32)
    freqs = np.outer(t, inv_freq)  # [T, 64]
    emb = np.concatenate([freqs, freqs], axis=-1)  # [T, D]
    cosT = np.ascontiguousarray(np.cos(emb).T.astype(np.float32))  # [D, T]
    # sign-folded sin: rotate_half contributes -q[d+64]*sin for d<64 and
    # +q[d-64]*sin for d>=64; the rotate DMA copies without negation, so
    # fold the sign into the sin table instead.
    sinT = np.sin(emb).T.astype(np.float32)
    sinT[: D // 2] = -sinT[: D // 2]
    sinT = np.ascontiguousarray(sinT)

    jj, kk = np.meshgrid(np.arange(128), np.arange(128), indexing="ij")
    tri = (jj <= kk).astype(np.float32)  # keep key_local <= query_local
    onesm = np.ones((128, 128), dtype=np.float32)
    ident = np.eye(128, dtype=np.float32)

    in_maps = []
    for c in range(N_CORES):
        h0 = c * HEADS_PER_CORE
        cols = slice(h0 * D, (h0 + HEADS_PER_CORE) * D)
        wq = w_qkv[:, 0 * C :][:, cols]
        wk = w_qkv[:, 1 * C :][:, cols]
        wv = w_qkv[:, 2 * C :][:, cols]
        wqkv_c = np.ascontiguousarray(np.concatenate([wq, wk, wv], axis=1))
        wout_c = np.ascontiguousarray(w_out[cols, :])
        in_maps.append(
            {
                "xT": xT,
                "wqkv": wqkv_c,
                "wout": wout_c,
                "cosT": cosT,
                "sinT": sinT,
                "tri": tri,
                "onesm": onesm,
                "ident": ident,
            }
        )
    return in_maps


def kernel(x, w_qkv, w_out):
    global LAST_RESULTS
    in_maps = _host_inputs(x, w_qkv, w_out)
    nc = build_nc()
    res = run_bass_kernel_spmd(
        nc, in_maps, core_ids=list(range(N_CORES)), trace=TRACE
    )
    LAST_RESULTS = res
    acc = np.zeros((BT, C), dtype=np.float64)
    for cr in res.results:
        acc += cr["out"].astype(np.float64)
    return acc.astype(np.float32).reshape(B, T, C)


# revision 26
# speedup vs baseline: 1.1788x; 1.0027x over previous
"""Causal self-attention (B=2, T=2048, C=2048, H=16, D=128) on 8 trn2 NeuronCores.

Sharding: tensor-parallel over heads — 2 heads per core. Each core computes
q/k/v projections for its heads, RoPE, causal softmax attention, and a
partial output projection (rows of w_out for its heads). Host sums the 8
partial outputs.

Device layout choices (see comments inline):
  * x is passed transposed (xT [C, B*T]) so the contraction dim C lands on
    SBUF partitions naturally.
  * q/k are kept transposed per head: [D=128 partitions, B*T free].
  * scores are computed transposed (S^T = k-block^T . q) so the second
    matmul (v^T @ P^T) needs no transpose; softmax sums over the partition
    axis are taken with an all-ones stationary matmul accumulated in PSUM.
  * no max-subtraction in softmax (scores are O(1), exp is safe in fp32).
  * all matmuls run as float32r (full-speed fp32 path on the PE).
"""

import math
import sys

import numpy as np

try:
    import concourse.bass as bass  # noqa: F401
except ImportError:
    sys.path.insert(0, "/opt/trn_rl_repo")

import concourse.bass as bass
import concourse.mybir as mybir
from concourse import bacc
import concourse.tile as tile
from concourse.bass_utils import run_bass_kernel_spmd

# Problem dims (hardcoded per harness contract)
B, T, C = 2, 2048, 2048
H = 16
D = 128
N_CORES = 8
HEADS_PER_CORE = H // N_CORES  # 2
ROPE_BASE = 10000.0

BT = B * T  # 4096
TQ = 512  # query tile (matmul moving free dim)
TK = 128  # key block (stationary operand columns)
KC = C // 128  # 16 contraction subtiles for the qkv projection

F32 = mybir.dt.float32
F32R = mybir.dt.float32r

# set by test harness to collect a profile
TRACE = False
LAST_RESULTS = None


def r(ap):
    """View an AP as float32r for full-rate fp32 matmul."""
    return ap.bitcast(F32R)


def _patch_act_tables():
    """Steer Bacc's ACT-table-set choice to the set holding BOTH Exp and Ln.

    By default Exp resolves to 'exp_and_others' and Ln to 'natural_log', so a
    kernel alternating them reloads tables (~1.3us) every switch. Hiding
    Exp/Ln from every other set forces 'natural_log_exp_and_others' — one
    load for the whole kernel. Table indices are untouched, so the emitted
    act_func_set_id stays consistent with act_info.json.
    """
    import concourse.bacc as bacc_mod

    if getattr(bacc_mod, "_act_tables_patched", False):
        return
    orig = bacc_mod.get_activation_tables

    def patched(arch):
        tables = orig(arch)
        if "natural_log_exp_and_others" in tables:
            exp = mybir.ActivationFunctionType.Exp
            ln = mybir.ActivationFunctionType.Ln
            for name, funcs in tables.items():
                if name != "natural_log_exp_and_others":
                    funcs.discard(exp)
                    funcs.discard(ln)
        return tables

    bacc_mod.get_activation_tables = patched
    bacc_mod._act_tables_patched = True


def build_nc():
    _patch_act_tables()
    nc = bacc.Bacc()

    xT = nc.dram_tensor("xT", [C, BT], F32, kind="ExternalInput").ap()
    wqkv = nc.dram_tensor("wqkv", [C, 6 * D], F32, kind="ExternalInput").ap()
    wout = nc.dram_tensor("wout", [2 * D, C], F32, kind="ExternalInput").ap()
    cosT = nc.dram_tensor("cosT", [D, T], F32, kind="ExternalInput").ap()
    sinT = nc.dram_tensor("sinT", [D, T], F32, kind="ExternalInput").ap()
    tri = nc.dram_tensor("tri", [128, 128], F32, kind="ExternalInput").ap()
    onesm = nc.dram_tensor("onesm", [128, 128], F32, kind="ExternalInput").ap()
    ident = nc.dram_tensor("ident", [128, 128], F32, kind="ExternalInput").ap()
    out = nc.dram_tensor("out", [BT, C], F32, kind="ExternalOutput").ap()

    scale = 1.0 / math.sqrt(D)
    NH = HEADS_PER_CORE
    NQT = T // TQ  # 4 query tiles per (b, h)
    NBT = BT // 128  # 32 row-blocks of BT

    with tile.TileContext(nc) as tc:
        with (
            tc.tile_pool(name="consts", bufs=1) as consts,
            tc.tile_pool(name="persist", bufs=1) as persist,
        ):
            # ---- constants resident for the whole kernel ----
            cos_sb = consts.tile([D, T], F32)
            nc.sync.dma_start(out=cos_sb, in_=cosT)
            sin_sb = consts.tile([D, T], F32)
            nc.sync.dma_start(out=sin_sb, in_=sinT)
            tri_sb = consts.tile([128, 128], F32)
            nc.sync.dma_start(out=tri_sb, in_=tri)
            ones_sb = consts.tile([128, 128], F32R)
            nc.sync.dma_start(out=ones_sb, in_=r(onesm))
            id_sb = consts.tile([128, 128], F32R)
            nc.sync.dma_start(out=id_sb, in_=r(ident))

            # ---- persistent activations ----
            qT_sb = persist.tile([D, NH, BT], F32R)  # q^T per head (RoPEd)
            kT_sb = persist.tile([D, NH, BT], F32R)  # k^T per head (RoPEd)
            # v in natural orientation: [128 (bt within block), head, block, d]
            vN_sb = persist.tile([128, NH, NBT, D], F32R)

            # =============================================================
            # Phase 1: qkv projection + RoPE + v transpose
            # =============================================================
            with (
                tc.tile_pool(name="p1_w", bufs=1) as p1_w,
                tc.tile_pool(name="p1_x", bufs=3) as p1_x,
                tc.tile_pool(name="p1_t", bufs=2) as p1_t,
                tc.tile_pool(name="p1_ps", bufs=1, space="PSUM") as p1_ps,
                tc.tile_pool(name="p1_ps2", bufs=2, space="PSUM") as p1_ps2,
            ):
                w_sb = p1_w.tile([128, KC, 6 * D], F32R)
                nc.sync.dma_start(
                    out=w_sb, in_=r(wqkv.rearrange("(ks p) c -> p ks c", p=128))
                )

                # v transposes for tile n are emitted at the start of tile
                # n+1 so they don't sit between a tile's last matmul and the
                # next tile's first (the vtile copies they depend on are long
                # done by then).
                pending_vt = []

                def emit_vtransposes():
                    for vtile, h, n_ in pending_vt:
                        for s4 in range(TQ // 128):
                            blk = n_ * (TQ // 128) + s4
                            tp = p1_ps2.tile(
                                [128, TQ], F32, tag="scratch_ps", name="tp"
                            )
                            nc.tensor.transpose(
                                r(tp[:, :128]),
                                vtile[:, s4 * 128 : (s4 + 1) * 128],
                                id_sb,
                            )
                            nc.vector.tensor_copy(
                                out=vN_sb[:, h, blk, :], in_=tp[:, :128]
                            )
                    pending_vt.clear()

                for n in range(BT // TQ):  # 8 column tiles of B*T
                    t0 = (n * TQ) % T  # position within the batch row
                    # one PSUM bank per output: q0,q1,k0,k1,v0,v1
                    ps = [
                        p1_ps.tile([128, TQ], F32, tag=f"qkvps{j}", name=f"qkvps{j}")
                        for j in range(6)
                    ]
                    emit_vtransposes()
                    for k in range(KC):
                        xn = p1_x.tile([128, TQ], F32R)
                        nc.sync.dma_start(
                            out=xn,
                            in_=r(xT[k * 128 : (k + 1) * 128, n * TQ : (n + 1) * TQ]),
                        )
                        for j in range(6):
                            nc.tensor.matmul(
                                ps[j],
                                w_sb[:, k, j * D : (j + 1) * D],
                                xn,
                                start=(k == 0),
                                stop=(k == KC - 1),
                            )
                    # q and k: RoPE, into qT_sb/kT_sb. rotate_half is a
                    # cross-partition move -> two SBUF->SBUF DMAs; sin_sb is
                    # sign-folded on the host so no negation is needed.
                    for h in range(NH):
                        for which, dest in ((0, qT_sb), (1, kT_sb)):
                            src_ps = ps[which * NH + h]
                            raw = p1_t.tile([128, TQ], F32, tag="raw", name="raw")
                            # alternate engines so psum eviction isn't serial
                            if (h + which) % 2 == 0:
                                nc.scalar.copy(out=raw, in_=src_ps)
                            else:
                                nc.vector.tensor_copy(out=raw, in_=src_ps)
                            rot = p1_t.tile([128, TQ], F32, tag="rot", name="rot")
                            nc.sync.dma_start(out=rot[0:64, :], in_=raw[64:128, :])
                            nc.sync.dma_start(out=rot[64:128, :], in_=raw[0:64, :])
                            qcos = p1_t.tile([128, TQ], F32, tag="qcos")
                            nc.vector.tensor_mul(
                                out=qcos, in0=raw, in1=cos_sb[:, t0 : t0 + TQ]
                            )
                            nc.vector.tensor_mul(
                                out=rot, in0=rot, in1=sin_sb[:, t0 : t0 + TQ]
                            )
                            nc.vector.tensor_add(
                                out=dest[:, h, n * TQ : (n + 1) * TQ],
                                in0=qcos,
                                in1=rot,
                            )
                    # v: copy to sbuf; transposes are deferred one tile
                    for h in range(NH):
                        vtile = p1_t.tile(
                            [128, TQ], F32R, tag="vtile", name="vtile", bufs=4
                        )
                        nc.scalar.copy(out=vtile, in_=ps[4 + h])
                        pending_vt.append((vtile, h, n))
                emit_vtransposes()

            # =============================================================
            # Phase 2+3 fused: attention per (b, qi) for both heads, then
            # immediately the output projection for those 512 rows. This
            # spreads the 32MB output DMA across the whole window instead of
            # a DMA-bound tail, and removes the phase transition stall.
            # =============================================================
            with (
                tc.tile_pool(name="p23_w", bufs=1) as p23_w,
                tc.tile_pool(name="p23_pt", bufs=6) as p23_pt,
                tc.tile_pool(name="p23_t", bufs=2) as p23_t,
                tc.tile_pool(name="p23_y", bufs=2) as p23_y,
                tc.tile_pool(name="p23_o", bufs=3) as p23_o,
                tc.tile_pool(name="p23_sps", bufs=2, space="PSUM") as p23_sps,
                tc.tile_pool(name="p23_yac", bufs=2, space="PSUM") as p23_yac,
                tc.tile_pool(name="p23_srp", bufs=2, space="PSUM") as p23_srp,
                tc.tile_pool(name="p23_ops", bufs=2, space="PSUM") as p23_ops,
            ):
                wout_sb = p23_w.tile([128, 2, C], F32R)
                nc.sync.dma_start(
                    out=wout_sb, in_=r(wout.rearrange("(s p) c -> p s c", p=128))
                )
                CT = min(TQ, C)  # output-projection column tile
                for b in range(B):
                    for qi in range(NQT):
                        i0 = qi * TQ
                        bt_i = b * T + i0
                        # rolling y^T for these 512 rows, both heads
                        yroll = p23_y.tile(
                            [D, NH, TQ], F32R, tag="yroll", name="yroll"
                        )
                        for h in range(NH):
                            yps = p23_yac.tile([128, TQ], F32, tag="yacc")
                            sps = p23_srp.tile([128, TQ], F32, tag="srep")
                            nblocks = qi * (TQ // 128) + (TQ // 128)
                            for jb in range(nblocks):
                                bt_j = b * T + jb * 128
                                mdiag = jb - qi * (TQ // 128)  # >=0 on diag band
                                c0 = 128 * mdiag if mdiag > 0 else 0
                                first = jb == 0
                                last = jb == nblocks - 1
                                sp = p23_sps.tile([128, TQ], F32, tag="sps")
                                nc.tensor.matmul(
                                    sp[:, c0:],
                                    kT_sb[:, h, bt_j : bt_j + 128],
                                    qT_sb[:, h, bt_i + c0 : bt_i + TQ],
                                    start=True,
                                    stop=True,
                                )
                                pt = p23_pt.tile([128, TQ], F32R, tag="pt")
                                nc.scalar.activation(
                                    out=pt[:, c0:],
                                    in_=sp[:, c0:],
                                    func=mybir.ActivationFunctionType.Exp,
                                    scale=scale,
                                )
                                if mdiag >= 0:
                                    nc.vector.tensor_mul(
                                        out=pt[:, c0 : c0 + 128],
                                        in0=pt[:, c0 : c0 + 128],
                                        in1=tri_sb,
                                    )
                                nc.tensor.matmul(
                                    yps[:, c0:],
                                    vN_sb[:, h, bt_j // 128, :],
                                    pt[:, c0:],
                                    start=first,
                                    stop=last,
                                )
                                nc.tensor.matmul(
                                    sps[:, c0:],
                                    ones_sb,
                                    pt[:, c0:],
                                    start=first,
                                    stop=last,
                                )
                            # normalize: y = y~ * exp(-ln(sum))
                            lg = p23_t.tile([128, TQ], F32, tag="lg")
                            nc.scalar.activation(
                                out=lg,
                                in_=sps,
                                func=mybir.ActivationFunctionType.Ln,
                            )
                            rr = p23_t.tile([128, TQ], F32, tag="rr")
                            nc.scalar.activation(
                                out=rr,
                                in_=lg,
                                func=mybir.ActivationFunctionType.Exp,
                                scale=-1.0,
                            )
                            nc.vector.tensor_mul(
                                out=yroll[:, h, :], in0=yps, in1=rr
                            )
                        # output projection for these 512 rows
                        for blk4 in range(TQ // 128):
                            bt0 = bt_i + blk4 * 128
                            otile = p23_o.tile([128, C], F32, tag="otile")
                            for ntc in range(C // CT):
                                ops = p23_ops.tile([128, CT], F32, tag="ops")
                                for h in range(NH):
                                    nc.tensor.matmul(
                                        ops,
                                        yroll[:, h, blk4 * 128 : (blk4 + 1) * 128],
                                        wout_sb[:, h, ntc * CT : (ntc + 1) * CT],
                                        start=(h == 0),
                                        stop=(h == NH - 1),
                                    )
                                if ntc % 2 == 0:
                                    nc.vector.tensor_copy(
                                        out=otile[:, ntc * CT : (ntc + 1) * CT],
                                        in_=ops,
                                    )
                                else:
                                    nc.scalar.copy(
                                        out=otile[:, ntc * CT : (ntc + 1) * CT],
                                        in_=ops,
                                    )
                            nc.sync.dma_start(
                                out=out[bt0 : bt0 + 128, :], in_=otile
                            )

    nc.finalize()
    return nc


def _host_inputs(x, w_qkv, w_out):
    """Host-side prep: transpose x, slice weights per core, RoPE tables."""
    x = np.asarray(x, dtype=np.float32)
    w_qkv = np.asarray(w_qkv, dtype=np.float32)
    w_out = np.asarray(w_out, dtype=np.float32)

    xT = np.ascontiguousarray(x.reshape(BT, C).T)  # [C, BT]

    # RoPE tables, transposed to [D, T]
    inv_freq = 1.0 / (
        ROPE_BASE ** (np.arange(0, D, 2, dtype=np.float32) / D)
    )  # [64]
    t = np.arange(T, dtype=np.float32)
    freqs = np.outer(t, inv_freq)  # [T, 64]
    emb = np.concatenate([freqs, freqs], axis=-1)  # [T, D]
    cosT = np.ascontiguousarray(np.cos(emb).T.astype(np.float32))  # [D, T]
    # sign-folded sin: rotate_half contributes -q[d+64]*sin for d<64 and
    # +q[d-64]*sin for d>=64; the rotate DMA copies without negation, so
    # fold the sign into the sin table instead.
    sinT = np.sin(emb).T.astype(np.float32)
    sinT[: D // 2] = -sinT[: D // 2]
    sinT = np.ascontiguousarray(sinT)

    jj, kk = np.meshgrid(np.arange(128), np.arange(128), indexing="ij")
    tri = (jj <= kk).astype(np.float32)  # keep key_local <= query_local
    onesm = np.ones((128, 128), dtype=np.float32)
    ident = np.eye(128, dtype=np.float32)

    in_maps = []
    for c in range(N_CORES):
        h0 = c * HEADS_PER_CORE
        cols = slice(h0 * D, (h0 + HEADS_PER_CORE) * D)
        wq = w_qkv[:, 0 * C :][:, cols]
        wk = w_qkv[:, 1 * C :][:, cols]
        wv = w_qkv[:, 2 * C :][:, cols]
        wqkv_c = np.ascontiguousarray(np.concatenate([wq, wk, wv], axis=1))
        wout_c = np.ascontiguousarray(w_out[cols, :])
        in_maps.append(
            {
                "xT": xT,
                "wqkv": wqkv_c,
                "wout": wout_c,
                "cosT": cosT,
                "sinT": sinT,
                "tri": tri,
                "onesm": onesm,
                "ident": ident,
            }
        )
    return in_maps


def kernel(x, w_qkv, w_out):
    global LAST_RESULTS
    in_maps = _host_inputs(x, w_qkv, w_out)
    nc = build_nc()
    res = run_bass_kernel_spmd(
        nc, in_maps, core_ids=list(range(N_CORES)), trace=TRACE
    )
    LAST_RESULTS = res
    acc = np.zeros((BT, C), dtype=np.float64)
    for cr in res.results:
        acc += cr["out"].astype(np.float64)
    return acc.astype(np.float32).reshape(B, T, C)


# revision 29
# speedup vs baseline: 1.2315x; 1.0447x over previous
"""Causal self-attention (B=2, T=2048, C=2048, H=16, D=128) on 8 trn2 NeuronCores.

Sharding: tensor-parallel over heads — 2 heads per core. Each core computes
q/k/v projections for its heads, RoPE, causal softmax attention, and a
partial output projection (rows of w_out for its heads). Host sums the 8
partial outputs.

Device layout choices (see comments inline):
  * x is passed transposed (xT [C, B*T]) so the contraction dim C lands on
    SBUF partitions naturally.
  * q/k are kept transposed per head: [D=128 partitions, B*T free].
  * scores are computed transposed (S^T = k-block^T . q) so the second
    matmul (v^T @ P^T) needs no transpose; softmax sums over the partition
    axis are taken with an all-ones stationary matmul accumulated in PSUM.
  * no max-subtraction in softmax (scores are O(1), exp is safe in fp32).
  * all matmuls run as float32r (full-speed fp32 path on the PE).
"""

import math
import sys

import numpy as np

try:
    import concourse.bass as bass  # noqa: F401
except ImportError:
    sys.path.insert(0, "/opt/trn_rl_repo")

import concourse.bass as bass
import concourse.mybir as mybir
from concourse import bacc
import concourse.tile as tile
from concourse.bass_utils import run_bass_kernel_spmd

# Problem dims (hardcoded per harness contract)
B, T, C = 2, 2048, 2048
H = 16
D = 128
N_CORES = 8
HEADS_PER_CORE = H // N_CORES  # 2
ROPE_BASE = 10000.0

BT = B * T  # 4096
TQ = 512  # query tile (matmul moving free dim)
TK = 128  # key block (stationary operand columns)
KC = C // 128  # 16 contraction subtiles for the qkv projection

F32 = mybir.dt.float32
F32R = mybir.dt.float32r

# set by test harness to collect a profile
TRACE = False
LAST_RESULTS = None


def r(ap):
    """View an AP as float32r for full-rate fp32 matmul."""
    return ap.bitcast(F32R)


def _patch_act_tables():
    """Steer Bacc's ACT-table-set choice to the set holding BOTH Exp and Ln.

    By default Exp resolves to 'exp_and_others' and Ln to 'natural_log', so a
    kernel alternating them reloads tables (~1.3us) every switch. Hiding
    Exp/Ln from every other set forces 'natural_log_exp_and_others' — one
    load for the whole kernel. Table indices are untouched, so the emitted
    act_func_set_id stays consistent with act_info.json.
    """
    import concourse.bacc as bacc_mod

    if getattr(bacc_mod, "_act_tables_patched", False):
        return
    orig = bacc_mod.get_activation_tables

    def patched(arch):
        tables = orig(arch)
        if "natural_log_exp_and_others" in tables:
            exp = mybir.ActivationFunctionType.Exp
            ln = mybir.ActivationFunctionType.Ln
            for name, funcs in tables.items():
                if name != "natural_log_exp_and_others":
                    funcs.discard(exp)
                    funcs.discard(ln)
        return tables

    bacc_mod.get_activation_tables = patched
    bacc_mod._act_tables_patched = True


def build_nc():
    _patch_act_tables()
    nc = bacc.Bacc()

    xT = nc.dram_tensor("xT", [C, BT], F32, kind="ExternalInput").ap()
    wqkv = nc.dram_tensor("wqkv", [C, 6 * D], F32, kind="ExternalInput").ap()
    wout = nc.dram_tensor("wout", [2 * D, C], F32, kind="ExternalInput").ap()
    cosT = nc.dram_tensor("cosT", [D, T], F32, kind="ExternalInput").ap()
    sinT = nc.dram_tensor("sinT", [D, T], F32, kind="ExternalInput").ap()
    tri = nc.dram_tensor("tri", [128, 128], F32, kind="ExternalInput").ap()
    onesm = nc.dram_tensor("onesm", [128, 128], F32, kind="ExternalInput").ap()
    ident = nc.dram_tensor("ident", [128, 128], F32, kind="ExternalInput").ap()
    out = nc.dram_tensor("out", [BT, C], F32, kind="ExternalOutput").ap()

    scale = 1.0 / math.sqrt(D)
    NH = HEADS_PER_CORE
    NQT = T // TQ  # 4 query tiles per (b, h)
    NBT = BT // 128  # 32 row-blocks of BT

    with tile.TileContext(nc) as tc:
        with (
            tc.tile_pool(name="consts", bufs=1) as consts,
            tc.tile_pool(name="persist", bufs=1) as persist,
        ):
            # ---- constants resident for the whole kernel ----
            cos_sb = consts.tile([D, T], F32)
            nc.sync.dma_start(out=cos_sb, in_=cosT)
            sin_sb = consts.tile([D, T], F32)
            nc.sync.dma_start(out=sin_sb, in_=sinT)
            tri_sb = consts.tile([128, 128], F32)
            nc.sync.dma_start(out=tri_sb, in_=tri)
            ones_sb = consts.tile([128, 128], F32R)
            nc.sync.dma_start(out=ones_sb, in_=r(onesm))
            id_sb = consts.tile([128, 128], F32R)
            nc.sync.dma_start(out=id_sb, in_=r(ident))

            # ---- persistent activations ----
            qT_sb = persist.tile([D, NH, BT], F32R)  # q^T per head (RoPEd)
            kT_sb = persist.tile([D, NH, BT], F32R)  # k^T per head (RoPEd)
            # v in natural orientation: [128 (bt within block), head, block, d]
            vN_sb = persist.tile([128, NH, NBT, D], F32R)

            # =============================================================
            # Phase 1: qkv projection + RoPE + v transpose
            # =============================================================
            with (
                tc.tile_pool(name="p1_w", bufs=1) as p1_w,
                tc.tile_pool(name="p1_x", bufs=4) as p1_x,
                tc.tile_pool(name="p1_t", bufs=2) as p1_t,
                tc.tile_pool(name="p1_ps", bufs=1, space="PSUM") as p1_ps,
                tc.tile_pool(name="p1_ps2", bufs=2, space="PSUM") as p1_ps2,
            ):
                # w_qkv is loaded per k-slice inside the first tile's k-loop
                # so the very first matmul only waits for one 384KB slice,
                # not the whole 6MB tensor.
                w_sb = p1_w.tile([128, KC, 6 * D], F32R)
                wqkv_r = r(wqkv.rearrange("(ks p) c -> p ks c", p=128))

                # v transposes for tile n are emitted at the start of tile
                # n+1 so they don't sit between a tile's last matmul and the
                # next tile's first (the vtile copies they depend on are long
                # done by then).
                pending_vt = []

                def emit_vtransposes():
                    for vtile, h, n_ in pending_vt:
                        for s4 in range(TQ // 128):
                            blk = n_ * (TQ // 128) + s4
                            tp = p1_ps2.tile(
                                [128, TQ], F32, tag="scratch_ps", name="tp"
                            )
                            nc.tensor.transpose(
                                r(tp[:, :128]),
                                vtile[:, s4 * 128 : (s4 + 1) * 128],
                                id_sb,
                            )
                            nc.vector.tensor_copy(
                                out=vN_sb[:, h, blk, :], in_=tp[:, :128]
                            )
                    pending_vt.clear()

                for n in range(BT // TQ):  # 8 column tiles of B*T
                    t0 = (n * TQ) % T  # position within the batch row
                    # one PSUM bank per output: q0,q1,k0,k1,v0,v1
                    ps = [
                        p1_ps.tile([128, TQ], F32, tag=f"qkvps{j}", name=f"qkvps{j}")
                        for j in range(6)
                    ]
                    emit_vtransposes()
                    for k in range(KC):
                        if n == 0:
                            nc.sync.dma_start(
                                out=w_sb[:, k, :], in_=wqkv_r[:, k, :]
                            )
                        xn = p1_x.tile([128, TQ], F32R)
                        nc.sync.dma_start(
                            out=xn,
                            in_=r(xT[k * 128 : (k + 1) * 128, n * TQ : (n + 1) * TQ]),
                        )
                        for j in range(6):
                            nc.tensor.matmul(
                                ps[j],
                                w_sb[:, k, j * D : (j + 1) * D],
                                xn,
                                start=(k == 0),
                                stop=(k == KC - 1),
                            )
                    # q and k: RoPE, into qT_sb/kT_sb. rotate_half is a
                    # cross-partition move -> two SBUF->SBUF DMAs; sin_sb is
                    # sign-folded on the host so no negation is needed.
                    for h in range(NH):
                        for which, dest in ((0, qT_sb), (1, kT_sb)):
                            src_ps = ps[which * NH + h]
                            raw = p1_t.tile([128, TQ], F32, tag="raw", name="raw")
                            # alternate engines so psum eviction isn't serial
                            if (h + which) % 2 == 0:
                                nc.scalar.copy(out=raw, in_=src_ps)
                            else:
                                nc.vector.tensor_copy(out=raw, in_=src_ps)
                            rot = p1_t.tile([128, TQ], F32, tag="rot", name="rot")
                            # gpsimd queue: keeps these off the xT-stream queue
                            nc.gpsimd.dma_start(out=rot[0:64, :], in_=raw[64:128, :])
                            nc.gpsimd.dma_start(out=rot[64:128, :], in_=raw[0:64, :])
                            qcos = p1_t.tile([128, TQ], F32, tag="qcos")
                            nc.vector.tensor_mul(
                                out=qcos, in0=raw, in1=cos_sb[:, t0 : t0 + TQ]
                            )
                            nc.vector.tensor_mul(
                                out=rot, in0=rot, in1=sin_sb[:, t0 : t0 + TQ]
                            )
                            nc.vector.tensor_add(
                                out=dest[:, h, n * TQ : (n + 1) * TQ],
                                in0=qcos,
                                in1=rot,
                            )
                    # v: copy to sbuf; transposes are deferred one tile
                    for h in range(NH):
                        vtile = p1_t.tile(
                            [128, TQ], F32R, tag="vtile", name="vtile", bufs=4
                        )
                        nc.scalar.copy(out=vtile, in_=ps[4 + h])
                        pending_vt.append((vtile, h, n))
                emit_vtransposes()

            # =============================================================
            # Phase 2+3 fused: attention per (b, qi) for both heads, then
            # immediately the output projection for those 512 rows. This
            # spreads the 32MB output DMA across the whole window instead of
            # a DMA-bound tail, and removes the phase transition stall.
            # =============================================================
            with (
                tc.tile_pool(name="p23_w", bufs=1) as p23_w,
                tc.tile_pool(name="p23_pt", bufs=6) as p23_pt,
                tc.tile_pool(name="p23_t", bufs=2) as p23_t,
                tc.tile_pool(name="p23_y", bufs=2) as p23_y,
                tc.tile_pool(name="p23_o", bufs=3) as p23_o,
                tc.tile_pool(name="p23_sps", bufs=2, space="PSUM") as p23_sps,
                tc.tile_pool(name="p23_yac", bufs=2, space="PSUM") as p23_yac,
                tc.tile_pool(name="p23_srp", bufs=2, space="PSUM") as p23_srp,
                tc.tile_pool(name="p23_ops", bufs=2, space="PSUM") as p23_ops,
            ):
                wout_sb = p23_w.tile([128, 2, C], F32R)
                nc.sync.dma_start(
                    out=wout_sb, in_=r(wout.rearrange("(s p) c -> p s c", p=128))
                )
                CT = min(TQ, C)  # output-projection column tile
                for b in range(B):
                    for qi in range(NQT):
                        i0 = qi * TQ
                        bt_i = b * T + i0
                        # rolling y^T for these 512 rows, both heads
                        yroll = p23_y.tile(
                            [D, NH, TQ], F32R, tag="yroll", name="yroll"
                        )
                        for h in range(NH):
                            yps = p23_yac.tile([128, TQ], F32, tag="yacc")
                            sps = p23_srp.tile([128, TQ], F32, tag="srep")
                            nblocks = qi * (TQ // 128) + (TQ // 128)
                            for jb in range(nblocks):
                                bt_j = b * T + jb * 128
                                mdiag = jb - qi * (TQ // 128)  # >=0 on diag band
                                c0 = 128 * mdiag if mdiag > 0 else 0
                                first = jb == 0
                                last = jb == nblocks - 1
                                sp = p23_sps.tile([128, TQ], F32, tag="sps")
                                nc.tensor.matmul(
                                    sp[:, c0:],
                                    kT_sb[:, h, bt_j : bt_j + 128],
                                    qT_sb[:, h, bt_i + c0 : bt_i + TQ],
                                    start=True,
                                    stop=True,
                                )
                                pt = p23_pt.tile([128, TQ], F32R, tag="pt")
                                nc.scalar.activation(
                                    out=pt[:, c0:],
                                    in_=sp[:, c0:],
                                    func=mybir.ActivationFunctionType.Exp,
                                    scale=scale,
                                )
                                if mdiag >= 0:
                                    nc.vector.tensor_mul(
                                        out=pt[:, c0 : c0 + 128],
                                        in0=pt[:, c0 : c0 + 128],
                                        in1=tri_sb,
                                    )
                                nc.tensor.matmul(
                                    yps[:, c0:],
                                    vN_sb[:, h, bt_j // 128, :],
                                    pt[:, c0:],
                                    start=first,
                                    stop=last,
                                )
                                nc.tensor.matmul(
                                    sps[:, c0:],
                                    ones_sb,
                                    pt[:, c0:],
                                    start=first,
                                    stop=last,
                                )
                            # normalize: y = y~ * exp(-ln(sum))
                            lg = p23_t.tile([128, TQ], F32, tag="lg")
                            nc.scalar.activation(
                                out=lg,
                                in_=sps,
                                func=mybir.ActivationFunctionType.Ln,
                            )
                            rr = p23_t.tile([128, TQ], F32, tag="rr")
                            nc.scalar.activation(
                                out=rr,
                                in_=lg,
                                func=mybir.ActivationFunctionType.Exp,
                                scale=-1.0,
                            )
                            nc.vector.tensor_mul(
                                out=yroll[:, h, :], in0=yps, in1=rr
                            )
                        # output projection for these 512 rows
                        for blk4 in range(TQ // 128):
                            bt0 = bt_i + blk4 * 128
                            otile = p23_o.tile([128, C], F32, tag="otile")
                            for ntc in range(C // CT):
                                ops = p23_ops.tile([128, CT], F32, tag="ops")
                                for h in range(NH):
                                    nc.tensor.matmul(
                                        ops,
                                        yroll[:, h, blk4 * 128 : (blk4 + 1) * 128],
                                        wout_sb[:, h, ntc * CT : (ntc + 1) * CT],
                                        start=(h == 0),
                                        stop=(h == NH - 1),
                                    )
                                if ntc % 2 == 0:
                                    nc.vector.tensor_copy(
                                        out=otile[:, ntc * CT : (ntc + 1) * CT],
                                        in_=ops,
                                    )
                                else:
                                    nc.scalar.copy(
                                        out=otile[:, ntc * CT : (ntc + 1) * CT],
                                        in_=ops,
                                    )
                            nc.sync.dma_start(
                                out=out[bt0 : bt0 + 128, :], in_=otile
                            )

    nc.finalize()
    return nc


def _host_inputs(x, w_qkv, w_out):
    """Host-side prep: transpose x, slice weights per core, RoPE tables."""
    x = np.asarray(x, dtype=np.float32)
    w_qkv = np.asarray(w_qkv, dtype=np.float32)
    w_out = np.asarray(w_out, dtype=np.float32)

    xT = np.ascontiguousarray(x.reshape(BT, C).T)  # [C, BT]

    # RoPE tables, transposed to [D, T]
    inv_freq = 1.0 / (
        ROPE_BASE ** (np.arange(0, D, 2, dtype=np.float32) / D)
    )  # [64]
    t = np.arange(T, dtype=np.float32)
    freqs = np.outer(t, inv_freq)  # [T, 64]
    emb = np.concatenate([freqs, freqs], axis=-1)  # [T, D]
    cosT = np.ascontiguousarray(np.cos(emb).T.astype(np.float32))  # [D, T]
    # sign-folded sin: rotate_half contributes -q[d+64]*sin for d<64 and
    # +q[d-64]*sin for d>=64; the rotate DMA copies without negation, so
    # fold the sign into the sin table instead.
    sinT = np.sin(emb).T.astype(np.float32)
    sinT[: D // 2] = -sinT[: D // 2]
    sinT = np.ascontiguousarray(sinT)

    jj, kk = np.meshgrid(np.arange(128), np.arange(128), indexing="ij")
    tri = (jj <= kk).astype(np.float32)  # keep key_local <= query_local
    onesm = np.ones((128, 128), dtype=np.float32)
    ident = np.eye(128, dtype=np.float32)

    in_maps = []
    for c in range(N_CORES):
        h0 = c * HEADS_PER_CORE
        cols = slice(h0 * D, (h0 + HEADS_PER_CORE) * D)
        wq = w_qkv[:, 0 * C :][:, cols]
        wk = w_qkv[:, 1 * C :][:, cols]
        wv = w_qkv[:, 2 * C :][:, cols]
        wqkv_c = np.ascontiguousarray(np.concatenate([wq, wk, wv], axis=1))
        wout_c = np.ascontiguousarray(w_out[cols, :])
        in_maps.append(
            {
                "xT": xT,
                "wqkv": wqkv_c,
                "wout": wout_c,
                "cosT": cosT,
                "sinT": sinT,
                "tri": tri,
                "onesm": onesm,
                "ident": ident,
            }
        )
    return in_maps


def kernel(x, w_qkv, w_out):
    global LAST_RESULTS
    in_maps = _host_inputs(x, w_qkv, w_out)
    nc = build_nc()
    res = run_bass_kernel_spmd(
        nc, in_maps, core_ids=list(range(N_CORES)), trace=TRACE
    )
    LAST_RESULTS = res
    acc = np.zeros((BT, C), dtype=np.float64)
    for cr in res.results:
        acc += cr["out"].astype(np.float64)
    return acc.astype(np.float32).reshape(B, T, C)


# revision 30
# speedup vs baseline: 1.3417x; 1.0895x over previous
"""Causal self-attention (B=2, T=2048, C=2048, H=16, D=128) on 8 trn2 NeuronCores.

Sharding: tensor-parallel over heads — 2 heads per core. Each core computes
q/k/v projections for its heads, RoPE, causal softmax attention, and a
partial output projection (rows of w_out for its heads). Host sums the 8
partial outputs.

Device layout choices (see comments inline):
  * x is passed transposed (xT [C, B*T]) so the contraction dim C lands on
    SBUF partitions naturally.
  * q/k are kept transposed per head: [D=128 partitions, B*T free].
  * scores are computed transposed (S^T = k-block^T . q) so the second
    matmul (v^T @ P^T) needs no transpose; softmax sums over the partition
    axis are taken with an all-ones stationary matmul accumulated in PSUM.
  * no max-subtraction in softmax (scores are O(1), exp is safe in fp32).
  * all matmuls run as float32r (full-speed fp32 path on the PE).
"""

import math
import sys

import numpy as np

try:
    import concourse.bass as bass  # noqa: F401
except ImportError:
    sys.path.insert(0, "/opt/trn_rl_repo")

import concourse.bass as bass
import concourse.mybir as mybir
from concourse import bacc
import concourse.tile as tile
from concourse.bass_utils import run_bass_kernel_spmd

# Problem dims (hardcoded per harness contract)
B, T, C = 2, 2048, 2048
H = 16
D = 128
N_CORES = 8
HEADS_PER_CORE = H // N_CORES  # 2
ROPE_BASE = 10000.0

BT = B * T  # 4096
TQ = 512  # query tile (matmul moving free dim)
TK = 128  # key block (stationary operand columns)
KC = C // 128  # 16 contraction subtiles for the qkv projection

F32 = mybir.dt.float32
F32R = mybir.dt.float32r

# set by test harness to collect a profile
TRACE = False
LAST_RESULTS = None


def r(ap):
    """View an AP as float32r for full-rate fp32 matmul."""
    return ap.bitcast(F32R)


def _patch_act_tables():
    """Steer Bacc's ACT-table-set choice to the set holding BOTH Exp and Ln.

    By default Exp resolves to 'exp_and_others' and Ln to 'natural_log', so a
    kernel alternating them reloads tables (~1.3us) every switch. Hiding
    Exp/Ln from every other set forces 'natural_log_exp_and_others' — one
    load for the whole kernel. Table indices are untouched, so the emitted
    act_func_set_id stays consistent with act_info.json.
    """
    import concourse.bacc as bacc_mod

    if getattr(bacc_mod, "_act_tables_patched", False):
        return
    orig = bacc_mod.get_activation_tables

    def patched(arch):
        tables = orig(arch)
        if "natural_log_exp_and_others" in tables:
            exp = mybir.ActivationFunctionType.Exp
            ln = mybir.ActivationFunctionType.Ln
            for name, funcs in tables.items():
                if name != "natural_log_exp_and_others":
                    funcs.discard(exp)
                    funcs.discard(ln)
        return tables

    bacc_mod.get_activation_tables = patched
    bacc_mod._act_tables_patched = True


def build_nc():
    _patch_act_tables()
    nc = bacc.Bacc()

    xT = nc.dram_tensor("xT", [C, BT], F32, kind="ExternalInput").ap()
    wqkv = nc.dram_tensor("wqkv", [C, 6 * D], F32, kind="ExternalInput").ap()
    wout = nc.dram_tensor("wout", [2 * D, C], F32, kind="ExternalInput").ap()
    cosT = nc.dram_tensor("cosT", [D, T], F32, kind="ExternalInput").ap()
    sinT = nc.dram_tensor("sinT", [D, T], F32, kind="ExternalInput").ap()
    tri = nc.dram_tensor("tri", [128, 128], F32, kind="ExternalInput").ap()
    onesm = nc.dram_tensor("onesm", [128, 128], F32, kind="ExternalInput").ap()
    ident = nc.dram_tensor("ident", [128, 128], F32, kind="ExternalInput").ap()
    out = nc.dram_tensor("out", [BT, C], F32, kind="ExternalOutput").ap()

    scale = 1.0 / math.sqrt(D)
    NH = HEADS_PER_CORE
    NQT = T // TQ  # 4 query tiles per (b, h)
    NBT = BT // 128  # 32 row-blocks of BT

    with tile.TileContext(nc) as tc:
        with (
            tc.tile_pool(name="consts", bufs=1) as consts,
            tc.tile_pool(name="persist", bufs=1) as persist,
        ):
            # ---- constants resident for the whole kernel ----
            cos_sb = consts.tile([D, T], F32)
            nc.sync.dma_start(out=cos_sb, in_=cosT)
            sin_sb = consts.tile([D, T], F32)
            nc.sync.dma_start(out=sin_sb, in_=sinT)
            tri_sb = consts.tile([128, 128], F32)
            nc.sync.dma_start(out=tri_sb, in_=tri)
            ones_sb = consts.tile([128, 128], F32R)
            nc.sync.dma_start(out=ones_sb, in_=r(onesm))
            id_sb = consts.tile([128, 128], F32R)
            nc.sync.dma_start(out=id_sb, in_=r(ident))

            # ---- persistent activations ----
            # split per n-tile so consumers depend only on the producing
            # tile's writes (one big tensor would serialize phase 2 behind
            # the LAST tile's RoPE chain)
            NT1 = BT // TQ
            qT_sb = [
                persist.tile([D, NH, TQ], F32R, name=f"qT{n}") for n in range(NT1)
            ]
            kT_sb = [
                persist.tile([D, NH, TQ], F32R, name=f"kT{n}") for n in range(NT1)
            ]
            # v natural: [128 (bt within block), head, sub-block, d] per n-tile
            vN_sb = [
                persist.tile([128, NH, TQ // 128, D], F32R, name=f"vN{n}")
                for n in range(NT1)
            ]

            # =============================================================
            # Phase 1: qkv projection + RoPE + v transpose
            # =============================================================
            with (
                tc.tile_pool(name="p1_w", bufs=1) as p1_w,
                tc.tile_pool(name="p1_x", bufs=6) as p1_x,
                tc.tile_pool(name="p1_t", bufs=2) as p1_t,
                tc.tile_pool(name="p1_ps", bufs=1, space="PSUM") as p1_ps,
                tc.tile_pool(name="p1_ps2", bufs=2, space="PSUM") as p1_ps2,
            ):
                # w_qkv is loaded per k-slice inside the first tile's k-loop
                # so the very first matmul only waits for one 384KB slice,
                # not the whole 6MB tensor.
                w_sb = p1_w.tile([128, KC, 6 * D], F32R)
                wqkv_r = r(wqkv.rearrange("(ks p) c -> p ks c", p=128))

                # v transposes for tile n are emitted at the start of tile
                # n+1 so they don't sit between a tile's last matmul and the
                # next tile's first (the vtile copies they depend on are long
                # done by then).
                pending_vt = []

                def emit_vtransposes():
                    for vtile, h, n_ in pending_vt:
                        for s4 in range(TQ // 128):
                            tp = p1_ps2.tile(
                                [128, TQ], F32, tag="scratch_ps", name="tp"
                            )
                            nc.tensor.transpose(
                                r(tp[:, :128]),
                                vtile[:, s4 * 128 : (s4 + 1) * 128],
                                id_sb,
                            )
                            nc.vector.tensor_copy(
                                out=vN_sb[n_][:, h, s4, :], in_=tp[:, :128]
                            )
                    pending_vt.clear()

                for n in range(BT // TQ):  # 8 column tiles of B*T
                    t0 = (n * TQ) % T  # position within the batch row
                    # one PSUM bank per output: q0,q1,k0,k1,v0,v1
                    ps = [
                        p1_ps.tile([128, TQ], F32, tag=f"qkvps{j}", name=f"qkvps{j}")
                        for j in range(6)
                    ]
                    emit_vtransposes()
                    for k in range(KC):
                        if n == 0:
                            nc.sync.dma_start(
                                out=w_sb[:, k, :], in_=wqkv_r[:, k, :]
                            )
                        xn = p1_x.tile([128, TQ], F32R)
                        nc.sync.dma_start(
                            out=xn,
                            in_=r(xT[k * 128 : (k + 1) * 128, n * TQ : (n + 1) * TQ]),
                        )
                        for j in range(6):
                            nc.tensor.matmul(
                                ps[j],
                                w_sb[:, k, j * D : (j + 1) * D],
                                xn,
                                start=(k == 0),
                                stop=(k == KC - 1),
                            )
                    # q and k: RoPE, into qT_sb/kT_sb. rotate_half is a
                    # cross-partition move -> two SBUF->SBUF DMAs; sin_sb is
                    # sign-folded on the host so no negation is needed.
                    for h in range(NH):
                        for which, dest in ((0, qT_sb), (1, kT_sb)):
                            src_ps = ps[which * NH + h]
                            raw = p1_t.tile([128, TQ], F32, tag="raw", name="raw")
                            # alternate engines so psum eviction isn't serial
                            if (h + which) % 2 == 0:
                                nc.scalar.copy(out=raw, in_=src_ps)
                            else:
                                nc.vector.tensor_copy(out=raw, in_=src_ps)
                            rot = p1_t.tile([128, TQ], F32, tag="rot", name="rot")
                            # gpsimd queue: keeps these off the xT-stream queue
                            nc.gpsimd.dma_start(out=rot[0:64, :], in_=raw[64:128, :])
                            nc.gpsimd.dma_start(out=rot[64:128, :], in_=raw[0:64, :])
                            qcos = p1_t.tile([128, TQ], F32, tag="qcos")
                            nc.vector.tensor_mul(
                                out=qcos, in0=raw, in1=cos_sb[:, t0 : t0 + TQ]
                            )
                            nc.vector.tensor_mul(
                                out=rot, in0=rot, in1=sin_sb[:, t0 : t0 + TQ]
                            )
                            nc.vector.tensor_add(
                                out=dest[n][:, h, :],
                                in0=qcos,
                                in1=rot,
                            )
                    # v: copy to sbuf; transposes are deferred one tile
                    for h in range(NH):
                        vtile = p1_t.tile(
                            [128, TQ], F32R, tag="vtile", name="vtile", bufs=3
                        )
                        nc.scalar.copy(out=vtile, in_=ps[4 + h])
                        pending_vt.append((vtile, h, n))
                emit_vtransposes()

            # =============================================================
            # Phase 2+3 fused: attention per (b, qi) for both heads, then
            # immediately the output projection for those 512 rows. This
            # spreads the 32MB output DMA across the whole window instead of
            # a DMA-bound tail, and removes the phase transition stall.
            # =============================================================
            with (
                tc.tile_pool(name="p23_w", bufs=1) as p23_w,
                tc.tile_pool(name="p23_pt", bufs=6) as p23_pt,
                tc.tile_pool(name="p23_t", bufs=2) as p23_t,
                tc.tile_pool(name="p23_y", bufs=2) as p23_y,
                tc.tile_pool(name="p23_o", bufs=3) as p23_o,
                tc.tile_pool(name="p23_sps", bufs=2, space="PSUM") as p23_sps,
                tc.tile_pool(name="p23_yac", bufs=2, space="PSUM") as p23_yac,
                tc.tile_pool(name="p23_srp", bufs=2, space="PSUM") as p23_srp,
                tc.tile_pool(name="p23_ops", bufs=2, space="PSUM") as p23_ops,
            ):
                wout_sb = p23_w.tile([128, 2, C], F32R)
                nc.sync.dma_start(
                    out=wout_sb, in_=r(wout.rearrange("(s p) c -> p s c", p=128))
                )
                CT = min(TQ, C)  # output-projection column tile
                for b in range(B):
                    for qi in range(NQT):
                        i0 = qi * TQ
                        bt_i = b * T + i0
                        # rolling y^T for these 512 rows, both heads
                        yroll = p23_y.tile(
                            [D, NH, TQ], F32R, tag="yroll", name="yroll"
                        )
                        for h in range(NH):
                            yps = p23_yac.tile([128, TQ], F32, tag="yacc")
                            sps = p23_srp.tile([128, TQ], F32, tag="srep")
                            nblocks = qi * (TQ // 128) + (TQ // 128)
                            for jb in range(nblocks):
                                bt_j = b * T + jb * 128
                                mdiag = jb - qi * (TQ // 128)  # >=0 on diag band
                                c0 = 128 * mdiag if mdiag > 0 else 0
                                first = jb == 0
                                last = jb == nblocks - 1
                                sp = p23_sps.tile([128, TQ], F32, tag="sps")
                                kn, ko = bt_j // TQ, bt_j % TQ
                                qn = bt_i // TQ
                                nc.tensor.matmul(
                                    sp[:, c0:],
                                    kT_sb[kn][:, h, ko : ko + 128],
                                    qT_sb[qn][:, h, c0:],
                                    start=True,
                                    stop=True,
                                )
                                pt = p23_pt.tile([128, TQ], F32R, tag="pt")
                                nc.scalar.activation(
                                    out=pt[:, c0:],
                                    in_=sp[:, c0:],
                                    func=mybir.ActivationFunctionType.Exp,
                                    scale=scale,
                                )
                                if mdiag >= 0:
                                    nc.vector.tensor_mul(
                                        out=pt[:, c0 : c0 + 128],
                                        in0=pt[:, c0 : c0 + 128],
                                        in1=tri_sb,
                                    )
                                nc.tensor.matmul(
                                    yps[:, c0:],
                                    vN_sb[kn][:, h, (bt_j % TQ) // 128, :],
                                    pt[:, c0:],
                                    start=first,
                                    stop=last,
                                )
                                nc.tensor.matmul(
                                    sps[:, c0:],
                                    ones_sb,
                                    pt[:, c0:],
                                    start=first,
                                    stop=last,
                                )
                            # normalize: y = y~ * exp(-ln(sum))
                            lg = p23_t.tile([128, TQ], F32, tag="lg")
                            nc.scalar.activation(
                                out=lg,
                                in_=sps,
                                func=mybir.ActivationFunctionType.Ln,
                            )
                            rr = p23_t.tile([128, TQ], F32, tag="rr")
                            nc.scalar.activation(
                                out=rr,
                                in_=lg,
                                func=mybir.ActivationFunctionType.Exp,
                                scale=-1.0,
                            )
                            nc.vector.tensor_mul(
                                out=yroll[:, h, :], in0=yps, in1=rr
                            )
                        # output projection for these 512 rows
                        for blk4 in range(TQ // 128):
                            bt0 = bt_i + blk4 * 128
                            otile = p23_o.tile([128, C], F32, tag="otile")
                            for ntc in range(C // CT):
                                ops = p23_ops.tile([128, CT], F32, tag="ops")
                                for h in range(NH):
                                    nc.tensor.matmul(
                                        ops,
                                        yroll[:, h, blk4 * 128 : (blk4 + 1) * 128],
                                        wout_sb[:, h, ntc * CT : (ntc + 1) * CT],
                                        start=(h == 0),
                                        stop=(h == NH - 1),
                                    )
                                if ntc % 2 == 0:
                                    nc.vector.tensor_copy(
                                        out=otile[:, ntc * CT : (ntc + 1) * CT],
                                        in_=ops,
                                    )
                                else:
                                    nc.scalar.copy(
                                        out=otile[:, ntc * CT : (ntc + 1) * CT],
                                        in_=ops,
                                    )
                            nc.sync.dma_start(
                                out=out[bt0 : bt0 + 128, :], in_=otile
                            )

    nc.finalize()
    return nc


def _host_inputs(x, w_qkv, w_out):
    """Host-side prep: transpose x, slice weights per core, RoPE tables."""
    x = np.asarray(x, dtype=np.float32)
    w_qkv = np.asarray(w_qkv, dtype=np.float32)
    w_out = np.asarray(w_out, dtype=np.float32)

    xT = np.ascontiguousarray(x.reshape(BT, C).T)  # [C, BT]

    # RoPE tables, transposed to [D, T]
    inv_freq = 1.0 / (
        ROPE_BASE ** (np.arange(0, D, 2, dtype=np.float32) / D)
    )  # [64]
    t = np.arange(T, dtype=np.float32)
    freqs = np.outer(t, inv_freq)  # [T, 64]
    emb = np.concatenate([freqs, freqs], axis=-1)  # [T, D]
    cosT = np.ascontiguousarray(np.cos(emb).T.astype(np.float32))  # [D, T]
    # sign-folded sin: rotate_half contributes -q[d+64]*sin for d<64 and
    # +q[d-64]*sin for d>=64; the rotate DMA copies without negation, so
    # fold the sign into the sin table instead.
    sinT = np.sin(emb).T.astype(np.float32)
    sinT[: D // 2] = -sinT[: D // 2]
    sinT = np.ascontiguousarray(sinT)

    jj, kk = np.meshgrid(np.arange(128), np.arange(128), indexing="ij")
    tri = (jj <= kk).astype(np.float32)  # keep key_local <= query_local
    onesm = np.ones((128, 128), dtype=np.float32)
    ident = np.eye(128, dtype=np.float32)

    in_maps = []
    for c in range(N_CORES):
        h0 = c * HEADS_PER_CORE
        cols = slice(h0 * D, (h0 + HEADS_PER_CORE) * D)
        wq = w_qkv[:, 0 * C :][:, cols]
        wk = w_qkv[:, 1 * C :][:, cols]
        wv = w_qkv[:, 2 * C :][:, cols]
        wqkv_c = np.ascontiguousarray(np.concatenate([wq, wk, wv], axis=1))
        wout_c = np.ascontiguousarray(w_out[cols, :])
        in_maps.append(
            {
                "xT": xT,
                "wqkv": wqkv_c,
                "wout": wout_c,
                "cosT": cosT,
                "sinT": sinT,
                "tri": tri,
                "onesm": onesm,
                "ident": ident,
            }
        )
    return in_maps


def kernel(x, w_qkv, w_out):
    global LAST_RESULTS
    in_maps = _host_inputs(x, w_qkv, w_out)
    nc = build_nc()
    res = run_bass_kernel_spmd(
        nc, in_maps, core_ids=list(range(N_CORES)), trace=TRACE
    )
    LAST_RESULTS = res
    acc = np.zeros((BT, C), dtype=np.float64)
    for cr in res.results:
        acc += cr["out"].astype(np.float64)
    return acc.astype(np.float32).reshape(B, T, C)
